# revision 1
# baseline (speedup 1.0000x reference)
"""LocalGNN_DB Trainium2 kernel: data-parallel over batch (8 cores, 1 traj each).

Single pass over t: S(t) streamed from HBM exactly once. Dual-layout diffusion:
  - natural diffusion  (states^T stationary, S moving)  -> u1,z2,z1 in [feat, node]
  - transposed diffusion (S stationary, states^T moving) -> u1T,z1T in [node, feat]
All matmuls in float32r (TF32-like, full PE rate at free-dim>=256), fp32 accumulate.
Layouts chosen so every compute access starts at a 32-aligned partition:
  stA cols: [y1T 0:64 | z1T 64:76 | xT 76:88]
  zc rows:  [x 0:12 | pad | z2 32:44 | z1 44:56 | pad | ones 64]  (H1e zero-padded to match)
"""
import sys
sys.path.insert(0, "/opt/trn_rl_repo")
import numpy as np

_CACHE = {}

B, T, N, G = 8, 64, 256, 12
F1, F2, R1, R2 = 64, 32, 32, 2


def _build():
    import concourse.tile as tile
    from concourse import bacc, mybir
    from concourse.tile import TileContext

    f32 = mybir.dt.float32
    import os
    f32r = mybir.dt.float32r if os.environ.get("MM_FP32R") else mybir.dt.float32
    Tanh = mybir.ActivationFunctionType.Tanh

    nc = bacc.Bacc("TRN2", target_bir_lowering=False, debug=False, num_devices=8)
    S_d = nc.dram_tensor("S", [T, N, N], f32, kind="ExternalInput")
    xn_d = nc.dram_tensor("xn", [T, G, N], f32, kind="ExternalInput")
    xT_d = nc.dram_tensor("xT", [T, N, G], f32, kind="ExternalInput")
    h1_d = nc.dram_tensor("H1e", [65, F1], f32, kind="ExternalInput")
    h2_d = nc.dram_tensor("H2e", [3 * F1 + 1, F2], f32, kind="ExternalInput")
    a1_d = nc.dram_tensor("A1e", [F2 + 1, R1], f32, kind="ExternalInput")
    a2_d = nc.dram_tensor("A2e", [R1 + 1, R2], f32, kind="ExternalInput")
    out_d = nc.dram_tensor("out", [T, R2, N], f32, kind="ExternalOutput")

    with TileContext(nc) as tc:
        with tc.tile_pool(name="consts", bufs=1) as consts, \
             tc.tile_pool(name="spool", bufs=4) as spool, \
             tc.tile_pool(name="states", bufs=3) as states, \
             tc.tile_pool(name="pnat", bufs=2, space="PSUM") as pnat, \
             tc.tile_pool(name="ptr", bufs=1, space="PSUM") as ptr, \
             tc.tile_pool(name="psm", bufs=2, space="PSUM") as psm:

            h1e = consts.tile([65, F1], f32r, tag="h1")
            h2a = consts.tile([128, F2], f32r, tag="h2a")
            h2b = consts.tile([65, F2], f32r, tag="h2b")
            a1e = consts.tile([F2 + 1, R1], f32r, tag="a1")
            a2e = consts.tile([R1 + 1, R2], f32r, tag="a2")
            nc.sync.dma_start(out=h1e, in_=h1_d[:, :].bitcast(f32r))
            nc.sync.dma_start(out=h2a, in_=h2_d[0:128, :].bitcast(f32r))
            nc.sync.dma_start(out=h2b, in_=h2_d[128:193, :].bitcast(f32r))
            nc.sync.dma_start(out=a1e, in_=a1_d[:, :].bitcast(f32r))
            nc.sync.dma_start(out=a2e, in_=a2_d[:, :].bitcast(f32r))

            stA_prev = [None, None]
            stB_prev = [None, None]

            for t in range(T):
                s0 = spool.tile([128, N], f32r, tag="s0", name="s0")
                s1 = spool.tile([128, N], f32r, tag="s1", name="s1")
                nc.sync.dma_start(out=s0, in_=S_d[t, 0:128, :].bitcast(f32r))
                nc.sync.dma_start(out=s1, in_=S_d[t, 128:256, :].bitcast(f32r))
                s_c = [s0, s1]

                stA = [states.tile([128, 88], f32r, tag=f"stA{c}", name=f"stA{c}")
                       for c in (0, 1)]
                stB = [states.tile([128, F1], f32r, tag=f"stB{c}", name=f"stB{c}")
                       for c in (0, 1)]
                zc = states.tile([65, N], f32r, tag="zc", name="zc")
                uca = states.tile([128, N], f32r, tag="uca", name="uca")
                ucb = states.tile([F1 + 1, N], f32r, tag="ucb", name="ucb")
                y2e = states.tile([F2 + 1, N], f32r, tag="y2e", name="y2e")
                ve = states.tile([F2 + 1, N], f32r, tag="ve", name="ve")

                for c in (0, 1):
                    nc.sync.dma_start(
                        out=stA[c][:, 76:88].bitcast(f32),
                        in_=xT_d[t, c * 128:(c + 1) * 128, :])
                nc.vector.memset(zc[0:32, :].bitcast(f32), 0.0)
                nc.sync.dma_start(out=zc[0:G, :], in_=xn_d[t, :, :].bitcast(f32r))
                nc.vector.memset(zc[64:65, :].bitcast(f32), 1.0)
                nc.vector.memset(ucb[64:65, :].bitcast(f32), 1.0)
                nc.vector.memset(y2e[32:33, :].bitcast(f32), 1.0)
                nc.vector.memset(ve[32:33, :].bitcast(f32), 1.0)

                if t == 0:
                    nc.vector.memset(zc[32:64, :].bitcast(f32), 0.0)
                    nc.vector.memset(uca[64:128, :].bitcast(f32), 0.0)
                    nc.vector.memset(ucb[0:64, :].bitcast(f32), 0.0)
                    for c in (0, 1):
                        nc.vector.memset(stA[c][:, 64:76].bitcast(f32), 0.0)
                        nc.vector.memset(stB[c][:, :].bitcast(f32), 0.0)
                else:
                    # natural diffusion -> pA rows: [u1 0:64 | z2 64:76 | z1 76:88]
                    pA = pnat.tile([88, N], f32, tag="natA", name="pA")
                    pB = pnat.tile([F1, N], f32, tag="natB", name="pB")
                    for c in (0, 1):
                        nc.tensor.matmul(out=pA[:, :], lhsT=stA_prev[c][:, :].bitcast(f32),
                                         rhs=s_c[c][:, :].bitcast(f32), start=(c == 0), stop=(c == 1))
                        nc.tensor.matmul(out=pB[:, :], lhsT=stB_prev[c][:, :].bitcast(f32),
                                         rhs=s_c[c][:, :].bitcast(f32), start=(c == 0), stop=(c == 1))
                    # transposed diffusion -> pT cols: [u1T 0:64 | z2T 64:76 | z1T 76:88]
                    pT = [ptr.tile([128, 88], f32, tag=f"pT{n}", name=f"pT{n}")
                          for n in (0, 1)]
                    for n in (0, 1):
                        for c in (0, 1):
                            nc.tensor.matmul(out=pT[n][:, :],
                                             lhsT=s_c[c][:, n * 128:(n + 1) * 128].bitcast(f32),
                                             rhs=stA_prev[c][:, :].bitcast(f32),
                                             start=(c == 0), stop=(c == 1))
                    nc.vector.memset(zc[32:64, :].bitcast(f32), 0.0)
                    nc.vector.tensor_copy(out=zc[32:56, :], in_=pA[64:88, :])
                    nc.vector.tensor_copy(out=uca[64:128, :], in_=pA[0:64, :])
                    nc.vector.tensor_copy(out=ucb[0:64, :], in_=pB[:, :])
                    for n in (0, 1):
                        nc.vector.tensor_copy(out=stA[n][:, 64:76].bitcast(f32), in_=pT[n][:, 76:88])
                        nc.vector.tensor_copy(out=stB[n][:, :].bitcast(f32), in_=pT[n][:, 0:64])

                # layer-1 taps (natural + transposed)
                p1 = psm.tile([F1, N], f32, tag="sm", name="p1")
                nc.tensor.matmul(out=p1[:, :], lhsT=h1e[:, :], rhs=zc[:, :],
                                 start=True, stop=True)
                nc.scalar.activation(out=uca[0:F1, :], in_=p1[:, :], func=Tanh)
                for n in (0, 1):
                    p1t = psm.tile([128, F1], f32, tag="sm", name="p1t")
                    nc.tensor.matmul(out=p1t[:, :], lhsT=zc[:, n * 128:(n + 1) * 128].bitcast(f32),
                                     rhs=h1e[:, :].bitcast(f32), start=True, stop=True)
                    nc.scalar.activation(out=stA[n][:, 0:F1].bitcast(f32), in_=p1t[:, :], func=Tanh)

                # layer-2 taps (natural only)
                p2 = psm.tile([F2, N], f32, tag="sm", name="p2")
                nc.tensor.matmul(out=p2[:, :], lhsT=h2a[:, :], rhs=uca[:, :],
                                 start=True, stop=False)
                nc.tensor.matmul(out=p2[:, :], lhsT=h2b[:, :], rhs=ucb[:, :],
                                 start=False, stop=True)
                nc.scalar.activation(out=y2e[0:F2, :], in_=p2[:, :], func=Tanh)

                # readout
                p3 = psm.tile([R1, N], f32, tag="sm", name="p3")
                nc.tensor.matmul(out=p3[:, :], lhsT=a1e[:, :], rhs=y2e[:, :],
                                 start=True, stop=True)
                nc.scalar.activation(out=ve[0:R1, :], in_=p3[:, :], func=Tanh)
                po = psm.tile([R2, N], f32, tag="sm", name="po")
                nc.tensor.matmul(out=po[:, :], lhsT=a2e[:, :], rhs=ve[:, :],
                                 start=True, stop=True)
                osb = states.tile([R2, N], f32, tag="osb", name="osb")
                nc.scalar.copy(out=osb[:, :], in_=po[:, :])
                nc.sync.dma_start(out=out_d[t, :, :], in_=osb[:, :])

                stA_prev, stB_prev = stA, stB

    nc.compile()
    return nc


def kernel(x, S, W1, b1, W2, b2, A1, c1, A2, c2):
    from concourse.bass_utils import run_bass_kernel_spmd

    if "nc" not in _CACHE:
        _CACHE["nc"] = _build()
    nc = _CACHE["nc"]

    x = np.asarray(x, dtype=np.float32)
    S = np.asarray(S, dtype=np.float32)
    W1 = np.asarray(W1, np.float32)
    W2 = np.asarray(W2, np.float32)
    # H1e rows: 0:12 = k0 (x), 32:44 = k2 (z2), 44:56 = k1 (z1), 64 = b1, rest 0
    H1e = np.zeros((65, F1), np.float32)
    H1e[0:G] = W1[:, 0, 0, :].T
    H1e[32:32 + G] = W1[:, 0, 2, :].T
    H1e[44:44 + G] = W1[:, 0, 1, :].T
    H1e[64] = np.asarray(b1, np.float32).reshape(F1)
    H2e = np.concatenate(
        [np.transpose(W2[:, 0], (1, 2, 0)).reshape(3 * F1, F2),
         np.asarray(b2, np.float32).reshape(1, F2)], axis=0)
    A1e = np.concatenate([np.asarray(A1, np.float32).T,
                          np.asarray(c1, np.float32).reshape(1, R1)], axis=0)
    A2e = np.concatenate([np.asarray(A2, np.float32).T,
                          np.asarray(c2, np.float32).reshape(1, R2)], axis=0)

    in_maps = []
    for b in range(B):
        xb = np.ascontiguousarray(x[b])
        in_maps.append({
            "S": np.ascontiguousarray(S[b, :, 0]),
            "xn": xb,
            "xT": np.ascontiguousarray(xb.transpose(0, 2, 1)),
            "H1e": H1e, "H2e": H2e, "A1e": A1e, "A2e": A2e,
        })
    _CACHE["in_maps"] = in_maps
    res = run_bass_kernel_spmd(nc, in_maps, core_ids=list(range(B)))
    return np.stack([res.results[b]["out"] for b in range(B)], axis=0)



# revision 3
# speedup vs baseline: 11665.3076x; 11665.3076x over previous
"""LocalGNN_DB Trainium2 kernel: data-parallel over batch (8 cores, 1 traj each).

Single pass over t: S(t) streamed from HBM exactly once. Dual-layout diffusion:
  - natural diffusion  (states^T stationary, S moving)  -> u1,z2,z1 in [feat, node]
  - transposed diffusion (S stationary, states^T moving) -> u1T,z1T in [node, feat]
All matmuls in float32r (TF32-like, full PE rate at free-dim>=256), fp32 accumulate.
Layouts chosen so every compute access starts at a 32-aligned partition:
  stA cols: [y1T 0:64 | z1T 64:76 | xT 76:88]
  zc rows:  [x 0:12 | pad | z2 32:44 | z1 44:56 | pad | ones 64]  (H1e zero-padded to match)
"""
import sys
sys.path.insert(0, "/opt/trn_rl_repo")
import numpy as np

_CACHE = {}

B, T, N, G = 8, 64, 256, 12
F1, F2, R1, R2 = 64, 32, 32, 2


def _build(reps=1):
    import concourse.tile as tile
    from concourse import bacc, mybir
    from concourse.tile import TileContext

    f32 = mybir.dt.float32
    import os
    f32r = mybir.dt.float32r if os.environ.get("MM_FP32R") else mybir.dt.float32
    Tanh = mybir.ActivationFunctionType.Tanh

    nc = bacc.Bacc("TRN2", target_bir_lowering=False, debug=False, num_devices=8)
    S_d = nc.dram_tensor("S", [T, N, N], f32, kind="ExternalInput")
    xn_d = nc.dram_tensor("xn", [T, G, N], f32, kind="ExternalInput")
    xT_d = nc.dram_tensor("xT", [T, N, G], f32, kind="ExternalInput")
    h1_d = nc.dram_tensor("H1e", [65, F1], f32, kind="ExternalInput")
    h2_d = nc.dram_tensor("H2e", [3 * F1 + 1, F2], f32, kind="ExternalInput")
    a1_d = nc.dram_tensor("A1e", [F2 + 1, R1], f32, kind="ExternalInput")
    a2_d = nc.dram_tensor("A2e", [R1 + 1, R2], f32, kind="ExternalInput")
    out_d = nc.dram_tensor("out", [T, R2, N], f32, kind="ExternalOutput")

    with TileContext(nc) as tc:
        with tc.tile_pool(name="consts", bufs=1) as consts, \
             tc.tile_pool(name="spool", bufs=4) as spool, \
             tc.tile_pool(name="states", bufs=3) as states, \
             tc.tile_pool(name="pnat", bufs=2, space="PSUM") as pnat, \
             tc.tile_pool(name="ptr", bufs=1, space="PSUM") as ptr, \
             tc.tile_pool(name="psm", bufs=2, space="PSUM") as psm:

            h1e = consts.tile([65, F1], f32r, tag="h1")
            h2a = consts.tile([128, F2], f32r, tag="h2a")
            h2b = consts.tile([65, F2], f32r, tag="h2b")
            a1e = consts.tile([F2 + 1, R1], f32r, tag="a1")
            a2e = consts.tile([R1 + 1, R2], f32r, tag="a2")
            nc.sync.dma_start(out=h1e, in_=h1_d[:, :].bitcast(f32r))
            nc.sync.dma_start(out=h2a, in_=h2_d[0:128, :].bitcast(f32r))
            nc.sync.dma_start(out=h2b, in_=h2_d[128:193, :].bitcast(f32r))
            nc.sync.dma_start(out=a1e, in_=a1_d[:, :].bitcast(f32r))
            nc.sync.dma_start(out=a2e, in_=a2_d[:, :].bitcast(f32r))

            stA_prev = [None, None]
            stB_prev = [None, None]

            for t in range(T * reps):
                t = t % T
                s0 = spool.tile([128, N], f32r, tag="s0", name="s0")
                s1 = spool.tile([128, N], f32r, tag="s1", name="s1")
                nc.sync.dma_start(out=s0, in_=S_d[t, 0:128, :].bitcast(f32r))
                nc.sync.dma_start(out=s1, in_=S_d[t, 128:256, :].bitcast(f32r))
                s_c = [s0, s1]

                stA = [states.tile([128, 88], f32r, tag=f"stA{c}", name=f"stA{c}")
                       for c in (0, 1)]
                stB = [states.tile([128, F1], f32r, tag=f"stB{c}", name=f"stB{c}")
                       for c in (0, 1)]
                zc = states.tile([65, N], f32r, tag="zc", name="zc")
                uca = states.tile([128, N], f32r, tag="uca", name="uca")
                ucb = states.tile([F1 + 1, N], f32r, tag="ucb", name="ucb")
                y2e = states.tile([F2 + 1, N], f32r, tag="y2e", name="y2e")
                ve = states.tile([F2 + 1, N], f32r, tag="ve", name="ve")

                for c in (0, 1):
                    nc.sync.dma_start(
                        out=stA[c][:, 76:88].bitcast(f32),
                        in_=xT_d[t, c * 128:(c + 1) * 128, :])
                nc.vector.memset(zc[0:32, :].bitcast(f32), 0.0)
                nc.sync.dma_start(out=zc[0:G, :], in_=xn_d[t, :, :].bitcast(f32r))
                nc.vector.memset(zc[64:65, :].bitcast(f32), 1.0)
                nc.vector.memset(ucb[64:65, :].bitcast(f32), 1.0)
                nc.vector.memset(y2e[32:33, :].bitcast(f32), 1.0)
                nc.vector.memset(ve[32:33, :].bitcast(f32), 1.0)

                if t == 0:
                    nc.vector.memset(zc[32:64, :].bitcast(f32), 0.0)
                    nc.vector.memset(uca[64:128, :].bitcast(f32), 0.0)
                    nc.vector.memset(ucb[0:64, :].bitcast(f32), 0.0)
                    for c in (0, 1):
                        nc.vector.memset(stA[c][:, 64:76].bitcast(f32), 0.0)
                        nc.vector.memset(stB[c][:, :].bitcast(f32), 0.0)
                else:
                    # natural diffusion -> pA rows: [u1 0:64 | z2 64:76 | z1 76:88]
                    pA = pnat.tile([88, N], f32, tag="natA", name="pA")
                    pB = pnat.tile([F1, N], f32, tag="natB", name="pB")
                    for c in (0, 1):
                        nc.tensor.matmul(out=pA[:, :], lhsT=stA_prev[c][:, :].bitcast(f32),
                                         rhs=s_c[c][:, :].bitcast(f32), start=(c == 0), stop=(c == 1))
                        nc.tensor.matmul(out=pB[:, :], lhsT=stB_prev[c][:, :].bitcast(f32),
                                         rhs=s_c[c][:, :].bitcast(f32), start=(c == 0), stop=(c == 1))
                    # transposed diffusion -> pT cols: [u1T 0:64 | z2T 64:76 | z1T 76:88]
                    pT = [ptr.tile([128, 88], f32, tag=f"pT{n}", name=f"pT{n}")
                          for n in (0, 1)]
                    for n in (0, 1):
                        for c in (0, 1):
                            nc.tensor.matmul(out=pT[n][:, :],
                                             lhsT=s_c[c][:, n * 128:(n + 1) * 128].bitcast(f32),
                                             rhs=stA_prev[c][:, :].bitcast(f32),
                                             start=(c == 0), stop=(c == 1))
                    nc.vector.memset(zc[32:64, :].bitcast(f32), 0.0)
                    nc.vector.tensor_copy(out=zc[32:56, :], in_=pA[64:88, :])
                    nc.vector.tensor_copy(out=uca[64:128, :], in_=pA[0:64, :])
                    nc.vector.tensor_copy(out=ucb[0:64, :], in_=pB[:, :])
                    for n in (0, 1):
                        nc.vector.tensor_copy(out=stA[n][:, 64:76].bitcast(f32), in_=pT[n][:, 76:88])
                        nc.vector.tensor_copy(out=stB[n][:, :].bitcast(f32), in_=pT[n][:, 0:64])

                # layer-1 taps (natural + transposed)
                p1 = psm.tile([F1, N], f32, tag="sm", name="p1")
                nc.tensor.matmul(out=p1[:, :], lhsT=h1e[:, :], rhs=zc[:, :],
                                 start=True, stop=True)
                nc.scalar.activation(out=uca[0:F1, :], in_=p1[:, :], func=Tanh)
                for n in (0, 1):
                    p1t = psm.tile([128, F1], f32, tag="sm", name="p1t")
                    nc.tensor.matmul(out=p1t[:, :], lhsT=zc[:, n * 128:(n + 1) * 128].bitcast(f32),
                                     rhs=h1e[:, :].bitcast(f32), start=True, stop=True)
                    nc.scalar.activation(out=stA[n][:, 0:F1].bitcast(f32), in_=p1t[:, :], func=Tanh)

                # layer-2 taps (natural only)
                p2 = psm.tile([F2, N], f32, tag="sm", name="p2")
                nc.tensor.matmul(out=p2[:, :], lhsT=h2a[:, :], rhs=uca[:, :],
                                 start=True, stop=False)
                nc.tensor.matmul(out=p2[:, :], lhsT=h2b[:, :], rhs=ucb[:, :],
                                 start=False, stop=True)
                nc.scalar.activation(out=y2e[0:F2, :], in_=p2[:, :], func=Tanh)

                # readout
                p3 = psm.tile([R1, N], f32, tag="sm", name="p3")
                nc.tensor.matmul(out=p3[:, :], lhsT=a1e[:, :], rhs=y2e[:, :],
                                 start=True, stop=True)
                nc.scalar.activation(out=ve[0:R1, :], in_=p3[:, :], func=Tanh)
                po = psm.tile([R2, N], f32, tag="sm", name="po")
                nc.tensor.matmul(out=po[:, :], lhsT=a2e[:, :], rhs=ve[:, :],
                                 start=True, stop=True)
                osb = states.tile([R2, N], f32, tag="osb", name="osb")
                nc.scalar.copy(out=osb[:, :], in_=po[:, :])
                nc.sync.dma_start(out=out_d[t, :, :], in_=osb[:, :])

                stA_prev, stB_prev = stA, stB

    nc.compile()
    return nc


def kernel(x, S, W1, b1, W2, b2, A1, c1, A2, c2):
    from concourse.bass_utils import run_bass_kernel_spmd

    if "nc" not in _CACHE:
        _CACHE["nc"] = _build()
    nc = _CACHE["nc"]

    x = np.asarray(x, dtype=np.float32)
    S = np.asarray(S, dtype=np.float32)
    W1 = np.asarray(W1, np.float32)
    W2 = np.asarray(W2, np.float32)
    # H1e rows: 0:12 = k0 (x), 32:44 = k2 (z2), 44:56 = k1 (z1), 64 = b1, rest 0
    H1e = np.zeros((65, F1), np.float32)
    H1e[0:G] = W1[:, 0, 0, :].T
    H1e[32:32 + G] = W1[:, 0, 2, :].T
    H1e[44:44 + G] = W1[:, 0, 1, :].T
    H1e[64] = np.asarray(b1, np.float32).reshape(F1)
    H2e = np.concatenate(
        [np.transpose(W2[:, 0], (1, 2, 0)).reshape(3 * F1, F2),
         np.asarray(b2, np.float32).reshape(1, F2)], axis=0)
    A1e = np.concatenate([np.asarray(A1, np.float32).T,
                          np.asarray(c1, np.float32).reshape(1, R1)], axis=0)
    A2e = np.concatenate([np.asarray(A2, np.float32).T,
                          np.asarray(c2, np.float32).reshape(1, R2)], axis=0)

    in_maps = []
    for b in range(B):
        xb = np.ascontiguousarray(x[b])
        in_maps.append({
            "S": np.ascontiguousarray(S[b, :, 0]),
            "xn": xb,
            "xT": np.ascontiguousarray(xb.transpose(0, 2, 1)),
            "H1e": H1e, "H2e": H2e, "A1e": A1e, "A2e": A2e,
        })
    _CACHE["in_maps"] = in_maps
    res = run_bass_kernel_spmd(nc, in_maps, core_ids=list(range(B)))
    return np.stack([res.results[b]["out"] for b in range(B)], axis=0)



# revision 4
# speedup vs baseline: 27271.8164x; 2.3379x over previous
"""LocalGNN_DB Trainium2 kernel v2: phase-batched over t, split-bf16 matmuls.

The K=3 taps make the t-1 'recurrence' a depth-2 finite window, so each
diffusion (z1,z2,u1,u2) is computed for all t with no serial chain; the
tap combines and readout run as wide batched matmuls over (T x N).

Precision: diffusion operands are hi/lo bf16 pairs (x = xh + xl), products
computed as 3 bf16 matmuls (xh*Sh + xl*Sh + xh*Sl) with f32 PSUM accumulate
-> ~2^-16 operand error at 3 cyc/row (vs fp32 4 cyc/row). Tap combines are
fp32 (small K), readouts plain bf16 (empirically sufficient).
"""
import sys
sys.path.insert(0, "/opt/trn_rl_repo")
import numpy as np

_CACHE = {}

B, T, N, G = 8, 64, 256, 12
F1, F2, R1, R2 = 64, 32, 32, 2
TB = 8                      # timesteps per block
NB = T // TB


def _build(reps=1):
    from concourse import bacc, mybir
    from concourse.tile import TileContext

    f32 = mybir.dt.float32
    bf16 = mybir.dt.bfloat16
    Tanh = mybir.ActivationFunctionType.Tanh

    nc = bacc.Bacc("TRN2", target_bir_lowering=False, debug=False, num_devices=8)
    SHL_d = nc.dram_tensor("SHL", [128, T, 2, 2 * N], bf16, kind="ExternalInput")
    XTH_d = nc.dram_tensor("XTH", [128, T, 2, G], bf16, kind="ExternalInput")
    XTL_d = nc.dram_tensor("XTL", [128, T, 2, G], bf16, kind="ExternalInput")
    XN_d = nc.dram_tensor("XN", [G, T, N], f32, kind="ExternalInput")
    H1_d = nc.dram_tensor("H1", [56, F1], f32, kind="ExternalInput")
    H2A_d = nc.dram_tensor("H2A", [128, F2], f32, kind="ExternalInput")
    H2B_d = nc.dram_tensor("H2B", [F1, F2], f32, kind="ExternalInput")
    A1_d = nc.dram_tensor("A1e", [F2, R1], bf16, kind="ExternalInput")
    A2_d = nc.dram_tensor("A2e", [R1, R2], bf16, kind="ExternalInput")
    C2_d = nc.dram_tensor("C2e", [1, R2], bf16, kind="ExternalInput")
    ID_d = nc.dram_tensor("IDe", [F1, F1], f32, kind="ExternalInput")
    B1_d = nc.dram_tensor("B1e", [F1, 1], f32, kind="ExternalInput")
    B2_d = nc.dram_tensor("B2e", [F2, 1], f32, kind="ExternalInput")
    C1_d = nc.dram_tensor("C1e", [R1, 1], f32, kind="ExternalInput")
    OUT_d = nc.dram_tensor("out", [2, 128, T, R2], f32, kind="ExternalOutput")

    with TileContext(nc) as tc:
        with tc.tile_pool(name="consts", bufs=1) as consts, \
             tc.tile_pool(name="spool", bufs=2) as spool, \
             tc.tile_pool(name="tpool", bufs=2) as tpool, \
             tc.tile_pool(name="npool", bufs=2) as npool, \
             tc.tile_pool(name="pz", bufs=1, space="PSUM") as pz, \
             tc.tile_pool(name="pyt", bufs=1, space="PSUM") as pyt, \
             tc.tile_pool(name="puu", bufs=1, space="PSUM") as puu, \
             tc.tile_pool(name="pc1", bufs=1, space="PSUM") as pc1, \
             tc.tile_pool(name="pc2", bufs=1, space="PSUM") as pc2, \
             tc.tile_pool(name="pro", bufs=1, space="PSUM") as pro:
            pzt = pz
            put = pyt

            h1 = consts.tile([56, F1], f32, tag="h1")
            h2a = consts.tile([128, F2], f32, tag="h2a")
            h2b = consts.tile([F1, F2], f32, tag="h2b")
            a1 = consts.tile([F2, R1], bf16, tag="a1")
            a2 = consts.tile([R1, R2], bf16, tag="a2")
            c2r = consts.tile([1, R2], bf16, tag="c2r")
            ident = consts.tile([F1, F1], f32, tag="ident")
            b1c = consts.tile([F1, 1], f32, tag="b1c")
            b2c = consts.tile([F2, 1], f32, tag="b2c")
            c1c = consts.tile([R1, 1], f32, tag="c1c")
            onesP = consts.tile([1, 128], bf16, tag="onesP")
            zxZ = consts.tile([128, 2 * 24], bf16, tag="zxZ")
            yuZ = consts.tile([128, 2 * 128], bf16, tag="yuZ")
            nc.sync.dma_start(out=h1, in_=H1_d[:, :])
            nc.sync.dma_start(out=h2a, in_=H2A_d[:, :])
            nc.sync.dma_start(out=h2b, in_=H2B_d[:, :])
            nc.sync.dma_start(out=a1, in_=A1_d[:, :])
            nc.sync.dma_start(out=a2, in_=A2_d[:, :])
            nc.sync.dma_start(out=c2r, in_=C2_d[:, :])
            nc.sync.dma_start(out=ident, in_=ID_d[:, :])
            nc.sync.dma_start(out=b1c, in_=B1_d[:, :])
            nc.sync.dma_start(out=b2c, in_=B2_d[:, :])
            nc.sync.dma_start(out=c1c, in_=C1_d[:, :])
            nc.vector.memset(onesP[:, :], 1.0)
            nc.vector.memset(zxZ[:, :], 0.0)
            nc.vector.memset(yuZ[:, :], 0.0)

            prev_zxT, prev_yuT = None, None

            for rep in range(reps):
                prev_zxT, prev_yuT = None, None
                for b in range(NB):
                    t0 = b * TB
                    sS = spool.tile([128, TB, 2, 2 * N], bf16, tag="sS", name="sS")
                    nc.sync.dma_start(out=sS, in_=SHL_d[:, t0:t0 + TB, :, :])
                    sSh = sS
                    sSl = None

                    # zxT cols: [xh 0:12 | z1h 12:24 | xl 24:36 | z1l 36:48]
                    zxT = tpool.tile([128, TB, 2, 48], bf16, tag="zxT", name="zxT")
                    nc.sync.dma_start(out=zxT[:, :, :, 0:G], in_=XTH_d[:, t0:t0 + TB, :, :])
                    nc.sync.dma_start(out=zxT[:, :, :, 24:24 + G], in_=XTL_d[:, t0:t0 + TB, :, :])
                    # yuT cols: [y1h 0:64 | u1h 64:128 | y1l 128:192 | u1l 192:256]
                    yuT = tpool.tile([128, TB, 2, 256], bf16, tag="yuT", name="yuT")

                    zc = npool.tile([56, TB, N], f32, tag="zc", name="zc")
                    uch = npool.tile([128, TB, N], f32, tag="uch", name="uch")
                    ucb = npool.tile([F1, TB, N], f32, tag="ucb", name="ucb")
                    y2n = npool.tile([F2, TB, N], bf16, tag="y2n", name="y2n")
                    vn = npool.tile([R1, TB, N], bf16, tag="vn", name="vn")
                    if rep == 0 and b < 2:
                        nc.vector.memset(zc[0:32, :, :], 0.0)
                    nc.sync.dma_start(out=zc[0:G, :, :], in_=XN_d[:, t0:t0 + TB, :])

                    def xstate(t, cols):
                        # [x | z1] (hi: cols=0, lo: cols=24) state at time t-1
                        if t == t0 and b == 0:
                            return zxZ[:, cols:cols + 24]
                        if t == t0:
                            return prev_zxT[:, TB - 1, :, cols:cols + 24]
                        return zxT[:, t - t0 - 1, :, cols:cols + 24]

                    def yustate(t, cols):
                        if t == t0 and b == 0:
                            return yuZ[:, cols:cols + 128]
                        if t == t0:
                            return prev_yuT[:, TB - 1, :, cols:cols + 128]
                        return yuT[:, t - t0 - 1, :, cols:cols + 128]

                    # --- z1T(t) = S(t)^T x(t-1)^T  (3-term split) -> zxT cols 12:24 / 36:48
                    z1tp = pzt.tile([128, TB, 2, G], f32, tag="z1tp", name="z1tp")
                    for tt in range(TB):
                        t = t0 + tt
                        for co in range(2):
                            first, k = True, 0
                            for (soff, xcol) in ((0, 0), (0, 24), (N, 0)):
                                for ci in range(2):
                                    k += 1
                                    nc.tensor.matmul(
                                        out=z1tp[:, tt, co, :],
                                        lhsT=sSh[:, tt, ci, soff + co * 128:soff + (co + 1) * 128],
                                        rhs=_xslice(xstate(t, xcol), ci, 0, G),
                                        start=first, stop=(k == 6))
                                    first = False
                    nc.scalar.copy(out=zxT[:, :, :, G:2 * G], in_=z1tp[:, :, :, :])
                    nc.vector.tensor_sub(
                        out=zxT[:, :, :, 24 + G:24 + 2 * G],
                        in0=z1tp[:, :, :, :],
                        in1=zxT[:, :, :, G:2 * G])

                    # --- zz(t) = [z1(t); z2(t)] natural (3-term split)
                    for tt2 in range(TB // 2):
                        zzp = pz.tile([24, 2, N], f32, tag="zzp", name="zzp")
                        for j in range(2):
                            t = t0 + tt2 * 2 + j
                            first, k = True, 0
                            for (soff, xcol) in ((0, 0), (0, 24), (N, 0)):
                                for ci in range(2):
                                    k += 1
                                    nc.tensor.matmul(
                                        out=zzp[:, j, :],
                                        lhsT=_xslice(xstate(t, xcol), ci, 0, 24),
                                        rhs=sSh[:, t - t0, ci, soff:soff + N],
                                        start=first, stop=(k == 6))
                                    first = False
                        nc.vector.tensor_copy(out=zc[32:56, 2 * tt2:2 * tt2 + 2, :],
                                              in_=zzp[:, :, :])

                    # --- layer-1 combine (fp32, batched) -> y1 f32 in uch[0:64]
                    for j in range(TB // 2):
                        p1 = pc1.tile([F1, 2, N], f32, tag="p1", name="p1")
                        nc.tensor.matmul(out=p1[:, :, :], lhsT=h1[:, :],
                                         rhs=zc[:, 2 * j:2 * j + 2, :],
                                         start=True, stop=True)
                        nc.scalar.activation(out=uch[0:F1, 2 * j:2 * j + 2, :],
                                             in_=p1[:, :, :], func=Tanh,
                                             bias=b1c[:, 0:1])

                    # --- y1T (PE transpose) then u1T (3-term), alternating per
                    # t-pair so u1T always sees the already-decomposed y1T(t-1)
                    for tt2 in range(TB // 2):
                        sl = slice(2 * tt2, 2 * tt2 + 2)
                        ytp = pyt.tile([128, 2, 2, F1], f32, tag="ytp", name="ytp")
                        for j in range(2):
                            t = t0 + tt2 * 2 + j
                            for c in range(2):
                                nc.tensor.transpose(
                                    out=ytp[:, j, c, :],
                                    in_=uch[0:F1, t - t0, c * 128:(c + 1) * 128],
                                    identity=ident[:, :])
                        nc.scalar.copy(out=yuT[:, sl, :, 0:F1], in_=ytp[:, :, :, :])
                        nc.vector.tensor_sub(out=yuT[:, sl, :, 128:128 + F1],
                                             in0=ytp[:, :, :, :],
                                             in1=yuT[:, sl, :, 0:F1])
                        u1tp = pyt.tile([128, 2, 2, F1], f32, tag="ytp", name="u1tp")
                        for j in range(2):
                            t = t0 + tt2 * 2 + j
                            for co in range(2):
                                first, k = True, 0
                                for (soff, ycol) in ((0, 0), (0, 128), (N, 0)):
                                    for ci in range(2):
                                        k += 1
                                        nc.tensor.matmul(
                                            out=u1tp[:, j, co, :],
                                            lhsT=sSh[:, t - t0, ci, soff + co * 128:soff + (co + 1) * 128],
                                            rhs=_xslice(yustate(t, ycol), ci, 0, F1),
                                            start=first, stop=(k == 6))
                                        first = False
                        nc.scalar.copy(out=yuT[:, sl, :, F1:128], in_=u1tp[:, :, :, :])
                        nc.vector.tensor_sub(out=yuT[:, sl, :, 192:256],
                                             in0=u1tp[:, :, :, :],
                                             in1=yuT[:, sl, :, F1:128])

                    # --- uu(t) = [u1(t); u2(t)] natural (3-term)
                    for tt2 in range(TB // 2):
                        uup = puu.tile([128, 2, N], f32, tag="uup", name="uup")
                        for j in range(2):
                            t = t0 + tt2 * 2 + j
                            first, k = True, 0
                            for (soff, ycol) in ((0, 0), (0, 128), (N, 0)):
                                for ci in range(2):
                                    k += 1
                                    nc.tensor.matmul(
                                        out=uup[:, j, :],
                                        lhsT=_xslice(yustate(t, ycol), ci, 0, 128),
                                        rhs=sSh[:, t - t0, ci, soff:soff + N],
                                        start=first, stop=(k == 6))
                                    first = False
                        sl = slice(2 * tt2, 2 * tt2 + 2)
                        nc.vector.tensor_copy(out=uch[F1:128, sl, :], in_=uup[0:F1, :, :])
                        nc.vector.tensor_copy(out=ucb[:, sl, :], in_=uup[F1:128, :, :])

                    # --- layer-2 combine (fp32, batched) -> y2 bf16
                    for j in range(TB // 2):
                        p2 = pc2.tile([F2, 2, N], f32, tag="p2", name="p2")
                        nc.tensor.matmul(out=p2[:, :, :], lhsT=h2a[:, :],
                                         rhs=uch[:, 2 * j:2 * j + 2, :],
                                         start=True, stop=False)
                        nc.tensor.matmul(out=p2[:, :, :], lhsT=h2b[:, :],
                                         rhs=ucb[:, 2 * j:2 * j + 2, :],
                                         start=False, stop=True)
                        nc.scalar.activation(out=y2n[:, 2 * j:2 * j + 2, :],
                                             in_=p2[:, :, :], func=Tanh,
                                             bias=b2c[:, 0:1])

                    # --- readout-1 (bf16, batched) -> v bf16
                    for j in range(TB // 2):
                        p3 = pc2.tile([R1, 2, N], f32, tag="p2", name="p3")
                        nc.tensor.matmul(out=p3[:, :, :], lhsT=a1[:, :],
                                         rhs=y2n[:, 2 * j:2 * j + 2, :],
                                         start=True, stop=True)
                        nc.scalar.activation(out=vn[:, 2 * j:2 * j + 2, :],
                                             in_=p3[:, :, :], func=Tanh,
                                             bias=c1c[:, 0:1])

                    # --- readout-2 per (t, c) into psum, then SBUF, then DMA out
                    outp = pro.tile([128, 2, TB, R2], f32, tag="outp", name="outp")
                    for tt in range(TB):
                        for c in range(2):
                            nc.tensor.matmul(out=outp[:, c, tt, :],
                                             lhsT=vn[:, tt, c * 128:(c + 1) * 128],
                                             rhs=a2[:, :], start=True, stop=False)
                            nc.tensor.matmul(out=outp[:, c, tt, :],
                                             lhsT=onesP[:, :], rhs=c2r[:, :],
                                             start=False, stop=True)
                    osb = npool.tile([128, 2, TB, R2], f32, tag="osb", name="osb")
                    nc.scalar.copy(out=osb[:, :, :, :], in_=outp[:, :, :, :])
                    for c in range(2):
                        nc.sync.dma_start(out=OUT_d[c, :, t0:t0 + TB, :],
                                          in_=osb[:, c, :, :])

                    prev_zxT, prev_yuT = zxT, yuT

    nc.compile()
    return nc


def _xslice(ap, ci, lo, width):
    """ap is either a zero const [128, W] or a state view [128, 2, W]; pick ci."""
    if len(ap.shape) == 2:
        return ap[:, lo:lo + width]
    return ap[:, ci, lo:lo + width]


def kernel(x, S, W1, b1, W2, b2, A1, c1, A2, c2):
    import ml_dtypes
    from concourse.bass_utils import run_bass_kernel_spmd

    if "nc" not in _CACHE:
        _CACHE["nc"] = _build()
    nc = _CACHE["nc"]
    bfdt = ml_dtypes.bfloat16

    def split(a):
        h = np.asarray(a, bfdt)
        l = np.asarray(a - h.astype(np.float32), bfdt)
        return h, l

    x = np.asarray(x, np.float32)
    S = np.asarray(S, np.float32)
    W1 = np.asarray(W1, np.float32)
    W2 = np.asarray(W2, np.float32)
    H1 = np.zeros((56, F1), np.float32)
    H1[0:G] = W1[:, 0, 0].T
    H1[32:32 + G] = W1[:, 0, 1].T
    H1[44:44 + G] = W1[:, 0, 2].T
    H2A = np.concatenate([W2[:, 0, 0].T, W2[:, 0, 1].T], 0)                # [128,32]
    H2B = W2[:, 0, 2].T                                                     # [64,32]
    A1e = np.asarray(np.asarray(A1, np.float32).T, bfdt)
    A2e = np.asarray(np.asarray(A2, np.float32).T, bfdt)
    C2e = np.asarray(np.asarray(c2, np.float32).reshape(1, R2), bfdt)
    IDe = np.eye(F1, dtype=np.float32)
    B1e = np.asarray(b1, np.float32).reshape(F1, 1)
    B2e = np.asarray(b2, np.float32).reshape(F2, 1)
    C1e = np.asarray(c1, np.float32).reshape(R1, 1)

    in_maps = []
    for bb in range(B):
        Sb = np.ascontiguousarray(S[bb, :, 0])            # [T,256,256]
        Sh, Sl = split(Sb)
        dev = lambda a: np.ascontiguousarray(
            a.reshape(T, 2, 128, a.shape[-1]).transpose(2, 0, 1, 3))
        xb = np.ascontiguousarray(x[bb])                   # [T,12,256]
        xT = np.ascontiguousarray(xb.transpose(0, 2, 1))   # [T,256,12]
        xTh, xTl = split(xT)
        in_maps.append({
            "SHL": dev(np.concatenate([Sh.astype(np.float32), Sl.astype(np.float32)], axis=-1).astype(Sh.dtype)),
            "XTH": dev(xTh), "XTL": dev(xTl),
            "XN": np.ascontiguousarray(xb.transpose(1, 0, 2)),
            "H1": H1, "H2A": H2A, "H2B": H2B,
            "A1e": A1e, "A2e": A2e, "C2e": C2e, "IDe": IDe,
            "B1e": B1e, "B2e": B2e, "C1e": C1e,
        })
    _CACHE["in_maps"] = in_maps
    res = run_bass_kernel_spmd(nc, in_maps, core_ids=list(range(B)))
    out = np.stack([res.results[bb]["out"] for bb in range(B)], axis=0)  # [B,2,128,T,2]
    return np.ascontiguousarray(out.transpose(0, 3, 4, 1, 2).reshape(B, T, R2, N))
